# revision 1
# baseline (speedup 1.0000x reference)
"""MLA/GQA attention kernel v3 for Trainium2, 8-core SPMD.

v2 + fp8e4m3 DoubleRow matmuls for everything upstream of the softmax:
the 1/sqrt(128) scale makes S tiny (sigma~0.11), so fp8 quantization of
x/W_q/W_k contributes only ~0.6% perturbation to softmax weights, while
DoubleRow runs the PE at 0.5 cycles/row and contracts two 128-chunks
(or two 32-latent halves) per instruction:
 - Q proj: fp8, contraction pairs of d-chunks      (131k -> 65k rows)
 - K proj: fp8, same                               ( 33k -> 16k rows)
 - S:      fp8, latent 64 split as 2x32            (262k -> 131k rows)
Downstream (exp, V, PV, O) stays bf16: the near-uniform softmax means
ctx is a heavily-cancelling sum and needs the precision.

Layouts:
 - kt8 [64, 2, T] fp8: slot s on partitions s*32:(s+1)*32, dim1 = latent
   half; q8[j] same per pair-tile.
 - v tiles [128, 192] bf16: [g0 | ones | g1]; slot s takes cols
   s*64..s*64+128 so PV lands ctx on partitions s*64:(s+1)*64 and a
   64x-replicated denominator on the other half.
"""
import sys

sys.path.insert(0, "/opt/trn_rl_repo")

import numpy as np
import ml_dtypes

import concourse.bass as bass  # noqa: F401
import concourse.mybir as mybir
import concourse.tile as tile
from concourse import bacc, bass_utils

D = 2048
T = 2048
NH = 16
NKV = 4
DH = 128
LAT = 64
B = 4
NCORE = 8
HQ = 8
NJ = 4
SCALE = 1.0 / np.sqrt(np.float32(DH))

NCC = D // 128
NCP = NCC // 2    # 8 cc pairs
NT = T // 128
NQ = T // 512

F32 = mybir.dt.float32
BF16 = mybir.dt.bfloat16
FP8 = mybir.dt.float8e4
EXP = mybir.ActivationFunctionType.Exp
DR = mybir.MatmulPerfMode.DoubleRow

_CACHE = {}


def _build(reps=1, phases="all", prelude=True):
    nc = bacc.Bacc("TRN2", target_bir_lowering=False, debug=False)
    xt_d = nc.dram_tensor("xt", [D, T], BF16, kind="ExternalInput").ap()
    wq_d = nc.dram_tensor("wq", [D, NJ * 128], BF16, kind="ExternalInput").ap()
    wk_d = nc.dram_tensor("wk", [D, 128], BF16, kind="ExternalInput").ap()
    wv_d = nc.dram_tensor("wv", [D, 128], BF16, kind="ExternalInput").ap()
    wo_d = nc.dram_tensor("wo", [NJ * 128, D], BF16, kind="ExternalInput").ap()
    out_d = nc.dram_tensor("out", [T, D], BF16, kind="ExternalOutput").ap()

    with tile.TileContext(nc) as tc:
      for rep in range(reps):
        R = f"r{rep}"
        with tc.tile_pool(name=f"persist{R}", bufs=1) as persist:
            xts = [persist.tile([128, T], BF16, tag=f"x{c}{R}", name=f"x{c}{R}")
                   for c in range(NCC)]
            wqs = [persist.tile([128, NJ * 128], BF16, tag=f"wq{c}{R}",
                                name=f"wq{c}{R}") for c in range(NCC)]
            wks = [persist.tile([128, 128], BF16, tag=f"wk{c}{R}",
                                name=f"wk{c}{R}") for c in range(NCC)]
            wvs = [persist.tile([128, 128], BF16, tag=f"wv{c}{R}", name=f"wv{c}{R}")
                   for c in range(NCC)]
            wos = [persist.tile([128, D], BF16, tag=f"wo{j}{R}", name=f"wo{j}{R}")
                   for j in range(NJ)]
            # q8[j]: [64, 2, T] fp8 — slot s on partitions s*32, latent
            # half on dim1; kt8 same
            q8s = [persist.tile([64, 2, T], FP8, tag=f"q8{j}{R}", name=f"q8{j}{R}")
                   for j in range(NJ)]
            kt8 = persist.tile([64, 2, T], FP8, tag=f"kt{R}", name=f"kt{R}")
            vts = [persist.tile([128, 192], BF16, tag=f"v{t}{R}", name=f"v{t}{R}")
                   for t in range(NT)]
            ctxp = [persist.tile([128, T], BF16, tag=f"c{j}{R}", name=f"c{j}{R}")
                    for j in range(NJ)]

            for c in range(NCC):
                nc.sync.dma_start(wks[c][:], wk_d[c * 128:(c + 1) * 128, :])
                nc.sync.dma_start(xts[c][:], xt_d[c * 128:(c + 1) * 128, :])
                nc.sync.dma_start(wvs[c][:], wv_d[c * 128:(c + 1) * 128, :])
                nc.sync.dma_start(wqs[c][:], wq_d[c * 128:(c + 1) * 128, :])
            for j in range(NJ):
                nc.sync.dma_start(wos[j][:], wo_d[j * 128:(j + 1) * 128, :])
            for t in range(NT):
                nc.vector.memset(vts[t][:], 1.0)

            with tc.tile_pool(name=f"actx{R}", bufs=2, space="PSUM") as acp, \
                 tc.tile_pool(name=f"aexp{R}", bufs=4) as aexp, \
                 tc.tile_pool(name=f"arec{R}", bufs=2) as arec, \
                 tc.tile_pool(name=f"aost{R}", bufs=2) as aost:

                def emit_head(qc, j, s, spool, wide):
                    ps_ctx = acp.tile([128, 512], F32, tag="ps_ctx",
                                      name=f"psc{qc}{j}{s}{R}")
                    step = 2 if wide else 1
                    for i in range(NT // step):
                        ps_s = spool.tile([128, 512 * step], F32, tag="ps_s",
                                          name=f"pss{qc}{j}{s}{i}{R}")
                        ex = aexp.tile([128, 512 * step], BF16,
                                       tag="expw" if wide else "expn",
                                       name=f"ex{qc}{j}{s}{i}{R}")
                        for u in range(step):
                            kc = step * i + u
                            nc.tensor.matmul(
                                ps_s[:, u * 512:(u + 1) * 512],
                                kt8[s * 32:(s + 1) * 32, :,
                                    kc * 128:(kc + 1) * 128],
                                q8s[j][s * 32:(s + 1) * 32, :,
                                       qc * 512:(qc + 1) * 512],
                                start=True, stop=True, perf_mode=DR)
                        nc.scalar.activation(ex[:], ps_s[:], EXP,
                                             scale=float(SCALE))
                        for u in range(step):
                            kc = step * i + u
                            nc.tensor.matmul(
                                ps_ctx[:],
                                vts[kc][:, s * 64:s * 64 + 128],
                                ex[:, u * 512:(u + 1) * 512],
                                start=(kc == 0), stop=(kc == NT - 1))
                    rec = arec.tile([128, 512], F32, tag="rec",
                                    name=f"rec{qc}{j}{s}{R}")
                    nc.vector.reciprocal(
                        rec[s * 64:(s + 1) * 64, :],
                        ps_ctx[(1 - s) * 64:(2 - s) * 64, :])
                    nc.vector.tensor_mul(
                        ctxp[j][s * 64:(s + 1) * 64,
                                qc * 512:(qc + 1) * 512],
                        ps_ctx[s * 64:(s + 1) * 64, :],
                        rec[s * 64:(s + 1) * 64, :])

                # ------------- Phase P (+ query-block-0 prelude) -----------
                pP_cm = tc.tile_pool(name=f"pP{R}", bufs=2, space="PSUM")
                pP = pP_cm.__enter__()
                sn_cm = tc.tile_pool(name=f"sn{R}", bufs=2, space="PSUM")
                sn = sn_cm.__enter__()
                # K proj (bf16) -> kt8 fp8 repack
                for half in range(2):
                    pk = pP.tile([128, 1024], F32, tag="pp",
                                 name=f"pk{half}{R}")
                    for c in range(NCC):
                        for f in range(2):
                            fo = half * 1024 + f * 512
                            nc.tensor.matmul(
                                pk[:, f * 512:(f + 1) * 512],
                                wks[c][:],
                                xts[c][:, fo:fo + 512],
                                start=(c == 0), stop=(c == NCC - 1))
                    for s in range(2):
                        for i in range(2):
                            nc.vector.tensor_copy(
                                kt8[s * 32:(s + 1) * 32, i,
                                    half * 1024:(half + 1) * 1024],
                                pk[s * 64 + i * 32:s * 64 + (i + 1) * 32, :])
                # V proj (bf16, natural orientation)
                for r in range(NT // 2):
                    pv = pP.tile([128, 1024], F32, tag="pp", name=f"pv{r}{R}")
                    for c in range(NCC):
                        for tl in range(2):
                            tg = 2 * r + tl
                            nc.tensor.matmul(
                                pv[:, tl * 512:tl * 512 + 128],
                                xts[c][:, tg * 128:(tg + 1) * 128], wvs[c][:],
                                start=(c == 0), stop=(c == NCC - 1))
                    for tl in range(2):
                        tg = 2 * r + tl
                        nc.vector.tensor_copy(
                            vts[tg][:, 0:64], pv[:, tl * 512:tl * 512 + 64])
                        nc.vector.tensor_copy(
                            vts[tg][:, 128:192],
                            pv[:, tl * 512 + 64:tl * 512 + 128])
                # Q proj (bf16) -> q8 fp8 repack, with qc0 prelude
                for j in range(NJ):
                    for half in range(2):
                        pq = pP.tile([128, 1024], F32, tag="pp",
                                     name=f"pq{j}{half}{R}")
                        for c in range(NCC):
                            for f in range(2):
                                fo = half * 1024 + f * 512
                                nc.tensor.matmul(
                                    pq[:, f * 512:(f + 1) * 512],
                                    wqs[c][:, j * 128:(j + 1) * 128],
                                    xts[c][:, fo:fo + 512],
                                    start=(c == 0), stop=(c == NCC - 1))
                        for s in range(2):
                            for i in range(2):
                                nc.vector.tensor_copy(
                                    q8s[j][s * 32:(s + 1) * 32, i,
                                           half * 1024:(half + 1) * 1024],
                                    pq[s * 64 + i * 32:s * 64 + (i + 1) * 32, :])
                    if prelude:
                        for s in range(2):
                            emit_head(0, j, s, sn, wide=False)

                if phases == "p":
                    for j in range(NJ):
                        nc.sync.dma_start(out_d[j * 128:(j + 1) * 128, :],
                                          ctxp[j][:] if prelude else xts[j][:])
                    sn_cm.__exit__(None, None, None)
                    pP_cm.__exit__(None, None, None)
                    continue

                sn_cm.__exit__(None, None, None)
                pP_cm.__exit__(None, None, None)

                # ---------------- Phase A+O ----------------
                with tc.tile_pool(name=f"as{R}", bufs=2, space="PSUM") as asp, \
                     tc.tile_pool(name=f"aoo{R}", bufs=2, space="PSUM") as aop:

                    pend = []
                    ostage = {}

                    def emit_o(tg, od):
                        if phases == "pa":
                            return
                        if od == 0:
                            ostage[tg] = aost.tile([128, D], BF16, tag="ost",
                                                   name=f"ost{tg}{R}")
                        oo = aop.tile([128, 512], F32, tag="oo",
                                      name=f"oo{tg}{od}{R}")
                        for j in range(NJ):
                            nc.tensor.matmul(
                                oo[:], ctxp[j][:, tg * 128:(tg + 1) * 128],
                                wos[j][:, od * 512:(od + 1) * 512],
                                start=(j == 0), stop=(j == NJ - 1))
                        st = ostage[tg]
                        nc.vector.tensor_copy(st[:, od * 512:(od + 1) * 512],
                                              oo[:])
                        if od == 3:
                            nc.sync.dma_start(
                                out_d[tg * 128:(tg + 1) * 128, :], st[:])
                            del ostage[tg]

                    qc0 = 1 if prelude else 0
                    if prelude:
                        pend.extend((tg, od) for tg in range(4)
                                    for od in range(4))
                    for qc in range(qc0, NQ):
                        for j in range(NJ):
                            for s in range(2):
                                emit_head(qc, j, s, asp, wide=True)
                                for _ in range(2):
                                    if pend:
                                        emit_o(*pend.pop(0))
                        pend.extend((tg, od)
                                    for tg in range(4 * qc, 4 * qc + 4)
                                    for od in range(4))
                    for g in pend:
                        emit_o(*g)
                    if phases == "pa":
                        for j in range(NJ):
                            nc.sync.dma_start(
                                out_d[j * 128:(j + 1) * 128, :], ctxp[j][:])

    nc.compile()
    return nc


LAST_RESULTS = None


def _prep_inputs(x, W_q, W_k, W_v, W_k_to_latent, W_v_to_latent,
                 W_k_from_latent, W_v_from_latent, W_o):
    x = np.asarray(x, np.float32)
    W_q = np.asarray(W_q, np.float32)
    W_k = np.asarray(W_k, np.float32)
    W_v = np.asarray(W_v, np.float32)
    W_ktl = np.asarray(W_k_to_latent, np.float32)
    W_vtl = np.asarray(W_v_to_latent, np.float32)
    W_kf = np.asarray(W_k_from_latent, np.float32)
    W_vf = np.asarray(W_v_from_latent, np.float32)
    W_o = np.asarray(W_o, np.float32)

    wq_eff = np.stack([W_q[:, h * DH:(h + 1) * DH] @ W_kf.T
                       for h in range(NH)], 1)          # [D, NH, LAT]
    wk_lat = np.stack([W_k[:, g * DH:(g + 1) * DH] @ W_ktl
                       for g in range(NKV)], 1)
    wv_lat = np.stack([W_v[:, g * DH:(g + 1) * DH] @ W_vtl
                       for g in range(NKV)], 1)
    wo_eff = np.stack([W_vf @ W_o[h * DH:(h + 1) * DH, :]
                       for h in range(NH)], 0)          # [NH, LAT, D]

    bf = ml_dtypes.bfloat16
    f8 = ml_dtypes.float8_e4m3
    in_maps = []
    for c in range(NCORE):
        b, p = c // 2, c % 2
        heads = [8 * p + j for j in range(HQ)]
        wq_core = np.concatenate(
            [np.concatenate([wq_eff[:, heads[j]], wq_eff[:, heads[j + 4]]], 1)
             for j in range(NJ)], 1)
        wk_core = np.concatenate([wk_lat[:, 2 * p], wk_lat[:, 2 * p + 1]], 1)
        wv_core = np.concatenate([wv_lat[:, 2 * p], wv_lat[:, 2 * p + 1]], 1)
        wo_core = np.concatenate(
            [np.concatenate([wo_eff[heads[j]], wo_eff[heads[j + 4]]], 0)
             for j in range(NJ)], 0)
        xt = np.ascontiguousarray(x[b].T)
        in_maps.append({
            "xt": xt.astype(bf),
            "wq": np.ascontiguousarray(wq_core).astype(bf),
            "wk": np.ascontiguousarray(wk_core).astype(bf),
            "wv": np.ascontiguousarray(wv_core).astype(bf),
            "wo": np.ascontiguousarray(wo_core).astype(bf),
        })
    return in_maps


def kernel(x, W_q, W_k, W_v, W_k_to_latent, W_v_to_latent,
           W_k_from_latent, W_v_from_latent, W_o):
    global LAST_RESULTS
    in_maps = _prep_inputs(x, W_q, W_k, W_v, W_k_to_latent, W_v_to_latent,
                           W_k_from_latent, W_v_from_latent, W_o)
    if "nc" not in _CACHE:
        _CACHE["nc"] = _build()
    nc = _CACHE["nc"]
    res = bass_utils.run_bass_kernel_spmd(nc, in_maps, core_ids=list(range(NCORE)))
    LAST_RESULTS = res
    out = np.empty((B, T, D), np.float32)
    for b in range(B):
        out[b] = (res.results[2 * b]["out"].astype(np.float32)
                  + res.results[2 * b + 1]["out"].astype(np.float32))
    return out



# revision 3
# speedup vs baseline: 1.9542x; 1.9542x over previous
"""MLA/GQA attention kernel v4 for Trainium2, 8-core SPMD.

Latent-space attention (W_k_from_latent absorbed into W_q, W_v_from_latent
+ W_o absorbed into an effective W_o), so S and PV contract over 64-dim
latents per head. 8 cores = 4 batches x 2 head-halves; each core does
8 heads (4 pairs j, pair = heads[j], heads[j+4] sharing kv slots 0/1).

v4 vs v3: drop fp8 DoubleRow. q/k latents stay fp8e4 but in plain
[128, T] layout (pair of heads / pair of kv slots stacked on the
partition axis), so:
 - projection psum -> fp8 repack is a single straight [128,1024] copy;
 - S matmuls are K=64 fp8 pairs at base partitions 0 and 64 writing
   adjacent psum banks (row-group concurrency; and FWL stays enabled);
 - one [128,1024] exp activation covers both heads of a pair;
 - PV pairs accumulate into one [128,1024] ctx psum tile:
   bank0 = head A [ctx 0:64 | denom 64:128], bank1 = head B [denom|ctx]
   via the v-tile [g0 | ones | g1] ones-column trick.
"""
import sys

sys.path.insert(0, "/opt/trn_rl_repo")

import numpy as np
import ml_dtypes

import concourse.bass as bass  # noqa: F401
import concourse.mybir as mybir
import concourse.tile as tile
from concourse import bacc, bass_utils

D = 2048
T = 2048
NH = 16
NKV = 4
DH = 128
LAT = 64
B = 4
NCORE = 8
HQ = 8
NJ = 4
SCALE = 1.0 / np.sqrt(np.float32(DH))

NCC = D // 128
NT = T // 128
NQ = T // 512

F32 = mybir.dt.float32
BF16 = mybir.dt.bfloat16
FP8 = mybir.dt.float8e4
EXP = mybir.ActivationFunctionType.Exp

_CACHE = {}


def _build(reps=1, phases="all", prelude=True):
    nc = bacc.Bacc("TRN2", target_bir_lowering=False, debug=False)
    xt_d = nc.dram_tensor("xt", [D, T], BF16, kind="ExternalInput").ap()
    wq_d = nc.dram_tensor("wq", [D, NJ * 128], BF16, kind="ExternalInput").ap()
    wk_d = nc.dram_tensor("wk", [D, 128], BF16, kind="ExternalInput").ap()
    wv_d = nc.dram_tensor("wv", [D, 128], BF16, kind="ExternalInput").ap()
    wo_d = nc.dram_tensor("wo", [NJ * 128, D], BF16, kind="ExternalInput").ap()
    out_d = nc.dram_tensor("out", [T, D], BF16, kind="ExternalOutput").ap()

    with tile.TileContext(nc) as tc:
      for rep in range(reps):
        R = f"r{rep}"
        with tc.tile_pool(name=f"persist{R}", bufs=1) as persist:
            xts = [persist.tile([128, T], BF16, tag=f"x{c}{R}", name=f"x{c}{R}")
                   for c in range(NCC)]
            wqs = [persist.tile([128, NJ * 128], BF16, tag=f"wq{c}{R}",
                                name=f"wq{c}{R}") for c in range(NCC)]
            wks = [persist.tile([128, 128], BF16, tag=f"wk{c}{R}",
                                name=f"wk{c}{R}") for c in range(NCC)]
            wvs = [persist.tile([128, 128], BF16, tag=f"wv{c}{R}", name=f"wv{c}{R}")
                   for c in range(NCC)]
            wos = [persist.tile([128, D], BF16, tag=f"wo{j}{R}", name=f"wo{j}{R}")
                   for j in range(NJ)]
            # q8s[j]: [128, T] fp8 — partitions = [head j lat 64 | head j+4
            # lat 64]; kt same with kv slots 0/1
            q8s = [persist.tile([128, T], FP8, tag=f"q8{j}{R}", name=f"q8{j}{R}")
                   for j in range(NJ)]
            kt8 = persist.tile([128, T], FP8, tag=f"kt{R}", name=f"kt{R}")
            vts = [persist.tile([128, 192], BF16, tag=f"v{t}{R}", name=f"v{t}{R}")
                   for t in range(NT)]
            ctxp = [persist.tile([128, T], BF16, tag=f"c{j}{R}", name=f"c{j}{R}")
                    for j in range(NJ)]

            for c in range(NCC):
                nc.sync.dma_start(wks[c][:], wk_d[c * 128:(c + 1) * 128, :])
                nc.sync.dma_start(xts[c][:], xt_d[c * 128:(c + 1) * 128, :])
                nc.sync.dma_start(wvs[c][:], wv_d[c * 128:(c + 1) * 128, :])
                nc.sync.dma_start(wqs[c][:], wq_d[c * 128:(c + 1) * 128, :])
            for j in range(NJ):
                nc.sync.dma_start(wos[j][:], wo_d[j * 128:(j + 1) * 128, :])
            for t in range(NT):
                nc.vector.memset(vts[t][:], 1.0)

            with tc.tile_pool(name=f"actx{R}", bufs=1, space="PSUM") as acp, \
                 tc.tile_pool(name=f"aexp{R}", bufs=5) as aexp, \
                 tc.tile_pool(name=f"arec{R}", bufs=2) as arec, \
                 tc.tile_pool(name=f"aost{R}", bufs=2) as aost:

                def emit_head_pair(qc, j, spool):
                    """S + exp + PV for head pair j over query block qc."""
                    ps_ctx = acp.tile([128, 1024], F32, tag="ps_ctx",
                                      name=f"psc{qc}{j}{R}")
                    q0 = qc * 512
                    for i in range(NT):
                        ps_s = spool.tile([128, 1024], F32, tag="ps_s",
                                          name=f"pss{qc}{j}{i}{R}")
                        ex = aexp.tile([128, 1024], BF16, tag="expw",
                                       name=f"ex{qc}{j}{i}{R}")
                        kcol = slice(i * 128, (i + 1) * 128)
                        nc.tensor.matmul(
                            ps_s[:, 0:512],
                            kt8[0:64, kcol],
                            q8s[j][0:64, q0:q0 + 512],
                            start=True, stop=True)
                        nc.tensor.matmul(
                            ps_s[:, 512:1024],
                            kt8[64:128, kcol],
                            q8s[j][64:128, q0:q0 + 512],
                            start=True, stop=True)
                        nc.scalar.activation(ex[:], ps_s[:], EXP,
                                             scale=float(SCALE))
                        nc.tensor.matmul(
                            ps_ctx[:, 0:512],
                            vts[i][:, 0:128],
                            ex[:, 0:512],
                            start=(i == 0), stop=(i == NT - 1))
                        nc.tensor.matmul(
                            ps_ctx[:, 512:1024],
                            vts[i][:, 64:192],
                            ex[:, 512:1024],
                            start=(i == 0), stop=(i == NT - 1))
                    rec = arec.tile([128, 512], F32, tag="rec",
                                    name=f"rec{qc}{j}{R}")
                    # head A: ctx parts 0:64 of bank0, denom parts 64:128
                    nc.vector.reciprocal(rec[0:64, :], ps_ctx[64:128, 0:512])
                    nc.vector.tensor_mul(
                        ctxp[j][0:64, q0:q0 + 512],
                        ps_ctx[0:64, 0:512], rec[0:64, :])
                    # head B: denom parts 0:64 of bank1, ctx parts 64:128
                    nc.vector.reciprocal(rec[64:128, :],
                                         ps_ctx[0:64, 512:1024])
                    nc.vector.tensor_mul(
                        ctxp[j][64:128, q0:q0 + 512],
                        ps_ctx[64:128, 512:1024], rec[64:128, :])

                # ------------- Phase P (+ query-block-0 prelude) -----------
                pP_cm = tc.tile_pool(name=f"pP{R}", bufs=2, space="PSUM")
                pP = pP_cm.__enter__()
                sn_cm = tc.tile_pool(name=f"sn{R}", bufs=1, space="PSUM")
                sn = sn_cm.__enter__()
                # K proj -> kt8 fp8 (direct layout: [slot0 lat | slot1 lat])
                for half in range(2):
                    pk = pP.tile([128, 1024], F32, tag="pp",
                                 name=f"pk{half}{R}")
                    for c in range(NCC):
                        for f in range(2):
                            fo = half * 1024 + f * 512
                            nc.tensor.matmul(
                                pk[:, f * 512:(f + 1) * 512],
                                wks[c][:],
                                xts[c][:, fo:fo + 512],
                                start=(c == 0), stop=(c == NCC - 1))
                    nc.vector.tensor_copy(
                        kt8[:, half * 1024:(half + 1) * 1024], pk[:])
                # V proj (bf16, natural orientation)
                for r in range(NT // 2):
                    pv = pP.tile([128, 1024], F32, tag="pp", name=f"pv{r}{R}")
                    for c in range(NCC):
                        for tl in range(2):
                            tg = 2 * r + tl
                            nc.tensor.matmul(
                                pv[:, tl * 512:tl * 512 + 128],
                                xts[c][:, tg * 128:(tg + 1) * 128], wvs[c][:],
                                start=(c == 0), stop=(c == NCC - 1))
                    for tl in range(2):
                        tg = 2 * r + tl
                        nc.vector.tensor_copy(
                            vts[tg][:, 0:64], pv[:, tl * 512:tl * 512 + 64])
                        nc.vector.tensor_copy(
                            vts[tg][:, 128:192],
                            pv[:, tl * 512 + 64:tl * 512 + 128])
                # Q proj per pair j -> q8 fp8 direct, with qc0 prelude
                for j in range(NJ):
                    for half in range(2):
                        pq = pP.tile([128, 1024], F32, tag="pp",
                                     name=f"pq{j}{half}{R}")
                        for c in range(NCC):
                            for f in range(2):
                                fo = half * 1024 + f * 512
                                nc.tensor.matmul(
                                    pq[:, f * 512:(f + 1) * 512],
                                    wqs[c][:, j * 128:(j + 1) * 128],
                                    xts[c][:, fo:fo + 512],
                                    start=(c == 0), stop=(c == NCC - 1))
                        nc.vector.tensor_copy(
                            q8s[j][:, half * 1024:(half + 1) * 1024], pq[:])
                    if prelude:
                        emit_head_pair(0, j, sn)

                if phases == "p":
                    for j in range(NJ):
                        nc.sync.dma_start(out_d[j * 128:(j + 1) * 128, :],
                                          ctxp[j][:] if prelude else xts[j][:])
                    sn_cm.__exit__(None, None, None)
                    pP_cm.__exit__(None, None, None)
                    continue

                sn_cm.__exit__(None, None, None)
                pP_cm.__exit__(None, None, None)

                # ---------------- Phase A+O ----------------
                with tc.tile_pool(name=f"as{R}", bufs=2, space="PSUM") as asp, \
                     tc.tile_pool(name=f"aoo{R}", bufs=2, space="PSUM") as aop:

                    pend = []
                    ostage = {}

                    def emit_o(tg, od):
                        if phases == "pa":
                            return
                        if od == 0:
                            ostage[tg] = aost.tile([128, D], BF16, tag="ost",
                                                   name=f"ost{tg}{R}")
                        oo = aop.tile([128, 512], F32, tag="oo",
                                      name=f"oo{tg}{od}{R}")
                        for j in range(NJ):
                            nc.tensor.matmul(
                                oo[:], ctxp[j][:, tg * 128:(tg + 1) * 128],
                                wos[j][:, od * 512:(od + 1) * 512],
                                start=(j == 0), stop=(j == NJ - 1))
                        st = ostage[tg]
                        nc.vector.tensor_copy(st[:, od * 512:(od + 1) * 512],
                                              oo[:])
                        if od == 3:
                            nc.sync.dma_start(
                                out_d[tg * 128:(tg + 1) * 128, :], st[:])
                            del ostage[tg]

                    qc0 = 1 if prelude else 0
                    if prelude:
                        pend.extend((tg, od) for tg in range(4)
                                    for od in range(4))
                    for qc in range(qc0, NQ):
                        for j in range(NJ):
                            emit_head_pair(qc, j, asp)
                            for _ in range(4):
                                if pend:
                                    emit_o(*pend.pop(0))
                        pend.extend((tg, od)
                                    for tg in range(4 * qc, 4 * qc + 4)
                                    for od in range(4))
                    for g in pend:
                        emit_o(*g)
                    if phases == "pa":
                        for j in range(NJ):
                            nc.sync.dma_start(
                                out_d[j * 128:(j + 1) * 128, :], ctxp[j][:])

    nc.compile()
    return nc


LAST_RESULTS = None


def _prep_inputs(x, W_q, W_k, W_v, W_k_to_latent, W_v_to_latent,
                 W_k_from_latent, W_v_from_latent, W_o):
    x = np.asarray(x, np.float32)
    W_q = np.asarray(W_q, np.float32)
    W_k = np.asarray(W_k, np.float32)
    W_v = np.asarray(W_v, np.float32)
    W_ktl = np.asarray(W_k_to_latent, np.float32)
    W_vtl = np.asarray(W_v_to_latent, np.float32)
    W_kf = np.asarray(W_k_from_latent, np.float32)
    W_vf = np.asarray(W_v_from_latent, np.float32)
    W_o = np.asarray(W_o, np.float32)

    wq_eff = np.stack([W_q[:, h * DH:(h + 1) * DH] @ W_kf.T
                       for h in range(NH)], 1)          # [D, NH, LAT]
    wk_lat = np.stack([W_k[:, g * DH:(g + 1) * DH] @ W_ktl
                       for g in range(NKV)], 1)
    wv_lat = np.stack([W_v[:, g * DH:(g + 1) * DH] @ W_vtl
                       for g in range(NKV)], 1)
    wo_eff = np.stack([W_vf @ W_o[h * DH:(h + 1) * DH, :]
                       for h in range(NH)], 0)          # [NH, LAT, D]

    bf = ml_dtypes.bfloat16
    in_maps = []
    for c in range(NCORE):
        b, p = c // 2, c % 2
        heads = [8 * p + j for j in range(HQ)]
        wq_core = np.concatenate(
            [np.concatenate([wq_eff[:, heads[j]], wq_eff[:, heads[j + 4]]], 1)
             for j in range(NJ)], 1)
        wk_core = np.concatenate([wk_lat[:, 2 * p], wk_lat[:, 2 * p + 1]], 1)
        wv_core = np.concatenate([wv_lat[:, 2 * p], wv_lat[:, 2 * p + 1]], 1)
        wo_core = np.concatenate(
            [np.concatenate([wo_eff[heads[j]], wo_eff[heads[j + 4]]], 0)
             for j in range(NJ)], 0)
        xt = np.ascontiguousarray(x[b].T)
        in_maps.append({
            "xt": xt.astype(bf),
            "wq": np.ascontiguousarray(wq_core).astype(bf),
            "wk": np.ascontiguousarray(wk_core).astype(bf),
            "wv": np.ascontiguousarray(wv_core).astype(bf),
            "wo": np.ascontiguousarray(wo_core).astype(bf),
        })
    return in_maps


def kernel(x, W_q, W_k, W_v, W_k_to_latent, W_v_to_latent,
           W_k_from_latent, W_v_from_latent, W_o):
    global LAST_RESULTS
    in_maps = _prep_inputs(x, W_q, W_k, W_v, W_k_to_latent, W_v_to_latent,
                           W_k_from_latent, W_v_from_latent, W_o)
    if "nc" not in _CACHE:
        _CACHE["nc"] = _build()
    nc = _CACHE["nc"]
    res = bass_utils.run_bass_kernel_spmd(nc, in_maps, core_ids=list(range(NCORE)))
    LAST_RESULTS = res
    out = np.empty((B, T, D), np.float32)
    for b in range(B):
        out[b] = (res.results[2 * b]["out"].astype(np.float32)
                  + res.results[2 * b + 1]["out"].astype(np.float32))
    return out
